# revision 1
# baseline (speedup 1.0000x reference)
"""Trainium2 Bass kernel for DeepseekAttention (T=4096, H=2048, 16 heads, d=128).

Tensor-parallel over heads: 8 NeuronCores x 2 heads each (SPMD, per-core inputs).
Host side: transpose hidden -> hidden^T fp16, slice w_qkv/w_o per core, and
precompute RoPE cos/sin tables + causal mask blocks. Per core:
  phase 1: Q^T/K^T = w^T x hidden^T in [d, T] layout (fp16 matmuls), RoPE via
           DVE with rotate-half done by SBUF-to-SBUF DMA partition swaps;
           V produced directly in [t, d] layout (hidden^T tiles stationary).
  phase 2: causal attention in S^T = K Q^T layout, 512-wide q-chunks:
           exp on ACT (no max subtraction needed: scores are O(1)), softmax
           denominator via ones-stationary matmul accumulated in PSUM,
           normalization via DVE reciprocal + GPSIMD partition_broadcast.
           Fully-masked q columns of diagonal k-tiles are skipped.
  phase 3: partial out = A @ w_o rows-slice (software-pipelined one chunk
           behind attention); fp16 partials summed across cores on the host.
"""

import numpy as np

import concourse.tile as tile
from concourse import bacc, mybir
from concourse.bass_utils import run_bass_kernel_spmd

T = 4096
HID = 2048
NHEADS = 16
HD = 128
NCORES = 8
HPC = NHEADS // NCORES        # 2 heads per core
FEAT = HPC * HD               # 256 per-core attention features
QKVF = 3 * FEAT               # 768 per-core qkv features
CH = 512                      # T-chunk width
NCH = T // CH                 # 8 chunks
KT = HID // 128               # 16 hidden k-tiles
FT = QKVF // 128              # 6 qkv feature tiles
SCALE = float(HD) ** -0.5
MASK_NEG = -30000.0

BF = mybir.dt.bfloat16
F16 = mybir.dt.float16
F32 = mybir.dt.float32


def _build_bass():
    nc = bacc.Bacc("TRN2", target_bir_lowering=False, debug=False,
                   num_devices=NCORES)

    hidT = nc.dram_tensor("hidT", [HID, T], F16, kind="ExternalInput").ap()
    wqkv = nc.dram_tensor("wqkv", [HID, QKVF], F16, kind="ExternalInput").ap()
    wo = nc.dram_tensor("wo", [FEAT, HID], F16, kind="ExternalInput").ap()
    cos2 = nc.dram_tensor("cos2", [128, T], F16, kind="ExternalInput").ap()
    sin2 = nc.dram_tensor("sin2", [128, T], F16, kind="ExternalInput").ap()
    masks = nc.dram_tensor("masks", [128, 4 * CH], F32, kind="ExternalInput").ap()
    out = nc.dram_tensor("out", [T, HID], F16, kind="ExternalOutput").ap()

    with tile.TileContext(nc) as tc:
        _emit(tc, hidT, wqkv, wo, cos2, sin2, masks, out)
    nc.compile()
    return nc


def _emit(tc, hidT, wqkv, wo, cos2, sin2, masks, out):
    nc = tc.nc
    from contextlib import ExitStack
    ctx = ExitStack()
    with ctx:
        const = ctx.enter_context(tc.tile_pool(name="const", bufs=1))
        hidp = ctx.enter_context(tc.tile_pool(name="hidp", bufs=2))
        rawp = ctx.enter_context(tc.tile_pool(name="rawp", bufs=6))
        ropep = ctx.enter_context(tc.tile_pool(name="ropep", bufs=4))
        persist = ctx.enter_context(tc.tile_pool(name="persist", bufs=1))
        ptp = ctx.enter_context(tc.tile_pool(name="ptp", bufs=8))
        smallp = ctx.enter_context(tc.tile_pool(name="smallp", bufs=3))
        stgp = ctx.enter_context(tc.tile_pool(name="stgp", bufs=2))
        # PSUM: 8 banks -> S/pb/wo 4, O+D 2, qkv 2 (released after phase 1,
        # its banks become the deeper late-attention O+D pool)
        psmm = ctx.enter_context(tc.tile_pool(name="psmm", bufs=4, space="PSUM"))
        pswo = psmm
        pso_cm = tc.tile_pool(name="pso", bufs=2, space="PSUM")
        pso = pso_cm.__enter__()
        psd = pso
        psqkv_cm = tc.tile_pool(name="psqkv", bufs=2, space="PSUM")
        psqkv = psqkv_cm.__enter__()

        # ---- constants ----
        ones_col = const.tile([128, 1], BF, tag="ones_col")
        nc.any.memset(ones_col[:], 1.0)
        wqkv_sb = const.tile([128, KT * QKVF], F16, tag="wqkv_sb")
        for kt in range(KT):
            nc.scalar.dma_start(wqkv_sb[:, kt * QKVF:(kt + 1) * QKVF],
                                wqkv[kt * 128:(kt + 1) * 128, :])
        cos_sb = const.tile([128, T], F16, tag="cos_sb")
        sin_sb = const.tile([128, T], F16, tag="sin_sb")
        mask_sb = const.tile([128, 4 * CH], F32, tag="mask_sb")
        nc.scalar.dma_start(cos_sb[:], cos2[:])
        nc.scalar.dma_start(sin_sb[:], sin2[:])
        nc.scalar.dma_start(mask_sb[:], masks[:])
        wo_sb = []
        for h in range(HPC):
            t = const.tile([128, HID], F16, tag=f"wo_sb{h}", name=f"wo_sb{h}")
            nc.scalar.dma_start(t[:], wo[h * 128:(h + 1) * 128, :])
            wo_sb.append(t)

        # ---- persistent activation tiles ----
        QTR = [[persist.tile([128, CH], F16, tag=f"qtr{h}_{c}", name=f"qtr{h}_{c}")
                for c in range(NCH)] for h in range(HPC)]
        KTR = [[persist.tile([128, CH], F16, tag=f"ktr{h}_{c}", name=f"ktr{h}_{c}")
                for c in range(NCH)] for h in range(HPC)]
        VV = persist.tile([128, HPC * T], BF, tag="vv", name="vv")
        AT = [[persist.tile([128, CH], F16, tag=f"at{h}_{c}", name=f"at{h}_{c}")
               for c in range(NCH)] for h in range(HPC)]

        # ================= phase 1: QKV^T projection + RoPE + V transpose ====
        for c in range(NCH):
            hid_sb = hidp.tile([128, KT * CH], F16, tag="hid", name=f"hid{c}")
            if c == 0:
                for kt in range(KT):
                    nc.sync.dma_start(
                        hid_sb[:, kt * CH:(kt + 1) * CH],
                        hidT[kt * 128:(kt + 1) * 128, c * CH:(c + 1) * CH])
            else:
                hid_v = hidT[:, c * CH:(c + 1) * CH].rearrange(
                    "(kt p) t -> p kt t", p=128)
                nc.sync.dma_start(
                    hid_sb[:].rearrange("p (kt t) -> p kt t", kt=KT), hid_v)

            def rope_evict(ps, ft):
                h = ft % 2
                raw = rawp.tile([128, CH], F16, tag="raw", name=f"raw{c}_{ft}")
                nc.scalar.copy(raw[:], ps[:])
                rot = ropep.tile([128, CH], F16, tag="rot", name=f"rot{c}_{ft}")
                nc.sync.dma_start(rot[0:64, :], raw[64:128, :])
                nc.sync.dma_start(rot[64:128, :], raw[0:64, :])
                ta = ropep.tile([128, CH], F16, tag="ta", name=f"ta{c}_{ft}")
                tb = ropep.tile([128, CH], F16, tag="tb", name=f"tb{c}_{ft}")
                csl = slice(c * CH, (c + 1) * CH)
                nc.vector.tensor_mul(ta[:], raw[:], cos_sb[:, csl])
                nc.vector.tensor_mul(tb[:], rot[:], sin_sb[:, csl])
                dst = QTR[h][c] if ft < 2 else KTR[h][c]
                nc.vector.tensor_add(dst[:], ta[:], tb[:])

            def qk_mm(ps, ft, kt):
                nc.tensor.matmul(
                    ps[:],
                    wqkv_sb[:, kt * QKVF + ft * 128: kt * QKVF + (ft + 1) * 128],
                    hid_sb[:, kt * CH:(kt + 1) * CH],
                    start=(kt == 0), stop=(kt == KT - 1))

            def v_mm(psv, j, kt):
                nc.tensor.matmul(
                    psv[:],
                    hid_sb[:, kt * CH + j * 128: kt * CH + (j + 1) * 128],
                    wqkv_sb[:, kt * QKVF + 512: kt * QKVF + 768],
                    start=(kt == 0), stop=(kt == KT - 1))

            if c == 0:
                # wavefront chunk 0: all 8 chains consume each (wqkv[kt],
                # hid[kt]) arrival together, hiding the cold-start DMA pacing.
                # Accumulators borrowed from pools that are idle at t=0.
                qk_ps = [psqkv.tile([128, CH], F32, tag="mmA", name="psqkv0_0"),
                         pso.tile([128, CH], F32, tag="o", name="psqkv0_1"),
                         psqkv.tile([128, CH], F32, tag="mmA", name="psqkv0_2"),
                         pso.tile([128, CH], F32, tag="o", name="psqkv0_3")]
                v_ps = [psmm.tile([128, 2 * 128], F32, tag="mm",
                                  name=f"psv0_{j}") for j in range(4)]
                for kt in range(KT):
                    for ft in range(4):
                        qk_mm(qk_ps[ft], ft, kt)
                    for j in range(4):
                        v_mm(v_ps[j], j, kt)
                for ft in range(4):
                    rope_evict(qk_ps[ft], ft)
                for j in range(4):
                    nc.scalar.copy(VV[:, j * 256:(j + 1) * 256], v_ps[j][:])
            else:
                # order: head-0's Q/K, then V, then head-1's Q/K -- head 0's
                # attention consumes its inputs first, head 1 has slack
                for ft in (0, 2, None, 1, 3):
                    if ft is None:
                        # V in [t, d] layout: lhsT = hidT tile, rhs = w_v cols
                        for j in range(4):
                            psv = psqkv.tile([128, 2 * 128], F32, tag="mmA",
                                             name=f"psv{c}_{j}")
                            for kt in range(KT):
                                v_mm(psv, j, kt)
                            kt_ = 4 * c + j
                            nc.scalar.copy(
                                VV[:, kt_ * 256:(kt_ + 1) * 256], psv[:])
                        continue
                    # Q^T (ft 0,1) and K^T (ft 2,3) in [d, T] layout -> RoPE
                    ps = psqkv.tile([128, CH], F32, tag="mmA",
                                    name=f"psqkv{c}_{ft}")
                    for kt in range(KT):
                        qk_mm(ps, ft, kt)
                    rope_evict(ps, ft)


        # ======= phase 2+3: causal attention interleaved with w_o, per chunk ==
        psod2 = None
        for c in range(NCH):
            nkt = 4 * (c + 1)
            if c == 4:
                psqkv_cm.__exit__(None, None, None)
                pso_cm.__exit__(None, None, None)
                psod2 = ctx.enter_context(
                    tc.tile_pool(name="psod2", bufs=4, space="PSUM"))
            def attn_s_exp(h, kt):
                r = kt - 4 * c
                qo = 128 * r if r > 0 else 0   # skip fully-masked q cols
                ps = psmm.tile([128, CH], F32, tag="mm", name=f"pss{h}_{c}_{kt}")
                nc.tensor.matmul(
                    ps[:, qo:],
                    KTR[h][kt // 4][:, (kt % 4) * 128:(kt % 4 + 1) * 128],
                    QTR[h][c][:, qo:],
                    start=True, stop=True)
                if r >= 0:
                    # mask only affects q in [128r, 128r+128) of this block
                    nc.vector.tensor_add(
                        ps[:, qo:qo + 128], ps[:, qo:qo + 128],
                        mask_sb[:, r * CH + qo:r * CH + qo + 128])
                pt = ptp.tile([128, CH], BF, tag="pt", name=f"pt{h}_{c}_{kt}")
                nc.scalar.activation(pt[:, qo:], ps[:, qo:],
                                     mybir.ActivationFunctionType.Exp,
                                     scale=SCALE)
                return kt, qo, pt

            def attn_pd_po(h, pend, pd, po):
                kt, qo, pt = pend
                nc.tensor.matmul(pd[:, qo:], ones_col[:], pt[:, qo:],
                                 start=(kt == 0), stop=(kt == nkt - 1))
                nc.tensor.matmul(po[:, qo:],
                                 VV[:, kt * 256 + h * 128: kt * 256 + (h + 1) * 128],
                                 pt[:, qo:],
                                 start=(kt == 0), stop=(kt == nkt - 1))

            def attn_kt_loop(h, pd, po):
                # keep pd/po two iterations behind S/exp so the PE queue
                # always has independent S work ahead of each dependent pair
                pending = []
                for kt in range(nkt):
                    pending.append(attn_s_exp(h, kt))
                    if len(pending) > 2:
                        attn_pd_po(h, pending.pop(0), pd, po)
                for pend in pending:
                    attn_pd_po(h, pend, pd, po)

            def attn_norm(h, pd, po):
                # normalize: AT = po * (1/pd), broadcast via GPSIMD (off PE)
                rcp = smallp.tile([1, CH], F32, tag="rcp", name=f"rcp{h}_{c}")
                nc.vector.reciprocal(rcp[:], pd[:])
                binv = smallp.tile([128, CH], F32, tag="binv", name=f"binv{h}_{c}")
                nc.gpsimd.partition_broadcast(binv[:], rcp[:])
                nc.vector.tensor_mul(AT[h][c][:], po[:], binv[:])

            for h in range(HPC):
                odp = pso if c < 4 else psod2
                tg = "o" if c < 4 else "o2"
                pd = odp.tile([1, CH], F32, tag=tg, name=f"pd{h}_{c}")
                po = odp.tile([128, CH], F32, tag=tg, name=f"po{h}_{c}")
                attn_kt_loop(h, pd, po)
                attn_norm(h, pd, po)

            # ---- output projection, one chunk behind attention ----
            for j in (range(4) if c >= 1 else []):
                _emit_wo_tile(nc, pswo, stgp, AT, wo_sb, out, c - 1, j)
        for j in range(4):
            _emit_wo_tile(nc, pswo, stgp, AT, wo_sb, out, NCH - 1, j)


_NC_CACHE = None


def _get_nc():
    global _NC_CACHE
    if _NC_CACHE is None:
        _NC_CACHE = _build_bass()
    return _NC_CACHE


def _f16(x):
    return np.ascontiguousarray(x).astype(np.float16)


def prepare_inputs(hidden_states, positions, w_qkv, w_o):
    """Host-side sharding/preprocessing -> list of per-core input maps."""
    hidden_states = np.asarray(hidden_states, dtype=np.float32)
    positions = np.asarray(positions)
    w_qkv = np.asarray(w_qkv, dtype=np.float32)
    w_o = np.asarray(w_o, dtype=np.float32)

    hidT_f16 = _f16(hidden_states.T)

    pos = positions.astype(np.float32)
    half = HD // 2
    inv_freq = 1.0 / (10000.0 ** (np.arange(half, dtype=np.float32) / half))
    freqs = np.outer(pos, inv_freq)          # [T, 64]
    cos = np.cos(freqs).T                    # [64, T]
    sin = np.sin(freqs).T
    cos2 = _f16(np.concatenate([cos, cos], axis=0))
    sin2 = _f16(np.concatenate([-sin, sin], axis=0))

    # causal masks for the 4 diagonal alignments: block r: 0 where 128r+k <= q
    k_idx = np.arange(128)[:, None]
    q_idx = np.arange(CH)[None, :]
    mblocks = [np.where(128 * r + k_idx <= q_idx, 0.0, MASK_NEG).astype(np.float32)
               for r in range(4)]
    masks_np = np.concatenate(mblocks, axis=1)

    in_maps = []
    for core in range(NCORES):
        heads = [HPC * core + i for i in range(HPC)]
        wq = [w_qkv[:, h * HD:(h + 1) * HD] for h in heads]
        wk = [w_qkv[:, FEAT * NCORES + h * HD:FEAT * NCORES + (h + 1) * HD]
              for h in heads]
        wv = [w_qkv[:, 2 * FEAT * NCORES + h * HD:2 * FEAT * NCORES + (h + 1) * HD]
              for h in heads]
        wqkv_core = _f16(np.concatenate(wq + wk + wv, axis=1))
        wo_core = _f16(np.concatenate(
            [w_o[h * HD:(h + 1) * HD, :] for h in heads], axis=0))
        in_maps.append({
            "hidT": hidT_f16,
            "wqkv": wqkv_core,
            "wo": wo_core,
            "cos2": cos2,
            "sin2": sin2,
            "masks": masks_np,
        })
    return in_maps


def kernel(hidden_states, positions, w_qkv, w_o):
    in_maps = prepare_inputs(hidden_states, positions, w_qkv, w_o)
    nc = _get_nc()
    try:
        res = run_bass_kernel_spmd(nc, in_maps, core_ids=list(range(NCORES)))
    except Exception:
        # transient device wedge from a prior crashed process: retry once
        res = run_bass_kernel_spmd(nc, in_maps, core_ids=list(range(NCORES)))
    acc = res.results[0]["out"].astype(np.float32)
    for i in range(1, NCORES):
        acc += res.results[i]["out"].astype(np.float32)
    return acc

def _emit_wo_tile(nc, pswo, stgp, AT, wo_sb, out, c, j):
    """w_o projection for T-tile tt = 4c+j: 4 n-chunks of 512 columns."""
    tt = 4 * c + j
    last = (c == NCH - 1)
    stg = stgp.tile([128, HID], F16, tag="stg", bufs=4, name=f"stg{tt}")
    for n in range(HID // CH):
        pw = pswo.tile([128, CH], F32, tag="mm", name=f"pw{tt}_{n}")
        for h in range(HPC):
            nc.tensor.matmul(
                pw[:],
                AT[h][c][:, j * 128:(j + 1) * 128],
                wo_sb[h][:, n * CH:(n + 1) * CH],
                start=(h == 0), stop=(h == HPC - 1))
        if n % 2 == 0:
            nc.vector.tensor_copy(stg[:, n * CH:(n + 1) * CH], pw[:])
        else:
            nc.scalar.copy(stg[:, n * CH:(n + 1) * CH], pw[:])
        if last and n == 1:
            # drain the first half early so the final DMA is half-sized
            eng = nc.sync if tt % 2 == 0 else nc.scalar
            eng.dma_start(out[tt * 128:(tt + 1) * 128, :HID // 2],
                          stg[:, :HID // 2])
    eng = nc.sync if tt % 2 == 0 else nc.scalar
    if last:
        eng.dma_start(out[tt * 128:(tt + 1) * 128, HID // 2:],
                      stg[:, HID // 2:])
    else:
        eng.dma_start(out[tt * 128:(tt + 1) * 128, :], stg[:])



# revision 4
# speedup vs baseline: 1.1832x; 1.1832x over previous
"""Trainium2 Bass kernel for DeepseekAttention (T=4096, H=2048, 16 heads, d=128).

Tensor-parallel over heads: 8 NeuronCores x 2 heads each (SPMD).

v1 design (fp8 DoubleRow + full phase interleaving):
  - QKV projection and w_o projection run as split-precision fp8 matmuls
    (hi=e4m3, lo=e5m2, 3 cross terms, lo*lo dropped) in DoubleRow perf mode,
    which contracts 2 k-tiles per instruction at 0.5 cycles/row: 0.75x the
    fp16 PE cost at ~0.2% error. Weights are pre-scaled by 32 on the host to
    center them in e4m3's range; the scale is undone via the exp() scale
    (Q.K picks up 32*32=1024) and a final host-side 1/256 on the output.
  - Attention S/PV matmuls stay fp16 (accuracy requires it).
  - Softmax denominator: exp tiles are accumulated on DVE/Pool into 3 fp16
    accumulators (bounded rounding depth), reduced across partitions with
    gpsimd partition_all_reduce -- no PE ones-matmuls, no extra PSUM bank.
  - Attention output is normalized and split to (e4m3 hi, e5m2 lo) on the
    fly for the fp8 w_o pass.
  - QKV projection for chunk c+1 is interleaved with attention for chunk c
    and w_o for chunk c-1, keeping PE continuously busy (p-state stays at
    2.4 GHz) and hiding ACT exp / DVE work under PE matmuls.
"""

import numpy as np
import ml_dtypes

import concourse.tile as tile
from concourse import bacc, bass_isa, mybir
from concourse.bass_utils import run_bass_kernel_spmd

T = 4096
HID = 2048
NHEADS = 16
HD = 128
NCORES = 8
HPC = NHEADS // NCORES        # 2 heads per core
FEAT = HPC * HD               # 256 per-core attention features
QKVF = 3 * FEAT               # 768 per-core qkv features
CH = 512                      # T-chunk width
NCH = T // CH                 # 8 chunks
KT = HID // 128               # 16 hidden k-tiles
NKP = KT // 2                 # 8 DoubleRow k-pairs
SCALE = float(HD) ** -0.5
WSCALE = 32.0                 # host pre-scale on w_qkv and w_o (e4m3 range)
ATSCALE = 8.0                 # scale on normalized attention output
MASK_NEG = -3.0e7             # scores carry a 1024x factor; must still kill exp

BF = mybir.dt.bfloat16
F16 = mybir.dt.float16
F32 = mybir.dt.float32
E4 = mybir.dt.float8e4
E5 = mybir.dt.float8e5
NE4 = ml_dtypes.float8_e4m3
NE5 = ml_dtypes.float8_e5m2
DR = mybir.MatmulPerfMode.DoubleRow
MUL = mybir.AluOpType.mult
ADD = mybir.AluOpType.add


def _build_bass():
    nc = bacc.Bacc("TRN2", target_bir_lowering=False, debug=False,
                   num_devices=NCORES)

    hid_hi = nc.dram_tensor("hid_hi", [128, KT, T], E4, kind="ExternalInput").ap()
    hid_lo = nc.dram_tensor("hid_lo", [128, KT, T], E5, kind="ExternalInput").ap()
    wq_hi = nc.dram_tensor("wq_hi", [128, KT, QKVF], E4, kind="ExternalInput").ap()
    wq_lo = nc.dram_tensor("wq_lo", [128, KT, QKVF], E5, kind="ExternalInput").ap()
    wo_hi = nc.dram_tensor("wo_hi", [128, HPC, HID], E4, kind="ExternalInput").ap()
    wo_lo = nc.dram_tensor("wo_lo", [128, HPC, HID], E5, kind="ExternalInput").ap()
    cos2 = nc.dram_tensor("cos2", [128, T], F16, kind="ExternalInput").ap()
    sin2 = nc.dram_tensor("sin2", [128, T], F16, kind="ExternalInput").ap()
    masks = nc.dram_tensor("masks", [128, 4 * CH], F32, kind="ExternalInput").ap()
    out = nc.dram_tensor("out", [T, HID], F16, kind="ExternalOutput").ap()

    with tile.TileContext(nc) as tc:
        _emit(tc, hid_hi, hid_lo, wq_hi, wq_lo, wo_hi, wo_lo, cos2, sin2,
              masks, out)
    nc.compile()
    return nc


def _emit(tc, hid_hi, hid_lo, wq_hi, wq_lo, wo_hi, wo_lo, cos2, sin2,
          masks, out):
    nc = tc.nc
    from contextlib import ExitStack
    ctx = ExitStack()
    with ctx:
        const = ctx.enter_context(tc.tile_pool(name="const", bufs=1))
        hidp = ctx.enter_context(tc.tile_pool(name="hidp", bufs=2))
        ropep = ctx.enter_context(tc.tile_pool(name="ropep", bufs=4))
        persist = ctx.enter_context(tc.tile_pool(name="persist", bufs=1))
        ptp = ctx.enter_context(tc.tile_pool(name="ptp", bufs=8))
        accp = ctx.enter_context(tc.tile_pool(name="accp", bufs=6))
        nrmp = ctx.enter_context(tc.tile_pool(name="nrmp", bufs=2))
        stgp = ctx.enter_context(tc.tile_pool(name="stgp", bufs=2))
        # PSUM: 8 banks: qkv 2, S 2, po 2, wo 2
        psqkv = ctx.enter_context(tc.tile_pool(name="psqkv", bufs=2, space="PSUM"))
        pss = ctx.enter_context(tc.tile_pool(name="pss", bufs=2, space="PSUM"))
        pso = ctx.enter_context(tc.tile_pool(name="pso", bufs=2, space="PSUM"))
        pswo = ctx.enter_context(tc.tile_pool(name="pswo", bufs=2, space="PSUM"))

        # ---- constants (scalar queue) ----
        wqh = const.tile([128, KT, QKVF], E4, tag="wqh")
        wql = const.tile([128, KT, QKVF], E5, tag="wql")
        nc.scalar.dma_start(wqh[:], wq_hi[:])
        nc.scalar.dma_start(wql[:], wq_lo[:])
        woh = const.tile([128, HPC, HID], E4, tag="woh")
        wol = const.tile([128, HPC, HID], E5, tag="wol")
        nc.scalar.dma_start(woh[:], wo_hi[:])
        nc.scalar.dma_start(wol[:], wo_lo[:])
        cos_sb = const.tile([128, T], F16, tag="cos_sb")
        sin_sb = const.tile([128, T], F16, tag="sin_sb")
        mask_sb = const.tile([128, 4 * CH], F32, tag="mask_sb")
        nc.scalar.dma_start(cos_sb[:], cos2[:])
        nc.scalar.dma_start(sin_sb[:], sin2[:])
        nc.scalar.dma_start(mask_sb[:], masks[:])

        # ---- persistent activation tiles ----
        QTR = [[persist.tile([128, CH], F16, tag=f"qtr{h}_{c}", name=f"qtr{h}_{c}")
                for c in range(NCH)] for h in range(HPC)]
        KTR = [[persist.tile([128, CH], F16, tag=f"ktr{h}_{c}", name=f"ktr{h}_{c}")
                for c in range(NCH)] for h in range(HPC)]
        VV = persist.tile([128, HPC * T], F16, tag="vv", name="vv")
        ATH = [persist.tile([128, HPC, CH], E4, tag=f"ath{c}", name=f"ath{c}")
               for c in range(NCH)]
        ATL = [persist.tile([128, HPC, CH], E5, tag=f"atl{c}", name=f"atl{c}")
               for c in range(NCH)]

        hid_tiles = {}

        def load_hid(c):
            hh = hidp.tile([128, KT, CH], E4, tag="hh", name=f"hh{c}")
            hl = hidp.tile([128, KT, CH], E5, tag="hl", name=f"hl{c}")
            csl = slice(c * CH, (c + 1) * CH)
            nc.gpsimd.dma_start(hh[:], hid_hi[:, :, csl])
            nc.gpsimd.dma_start(hl[:], hid_lo[:, :, csl])
            hid_tiles[c] = (hh, hl)

        deferred_rope = []

        def rope_evict(ps, ft, c):
            # ft: 0=q.h0 1=q.h1 2=k.h0 3=k.h1. ACT copy + partition-swap DMA
            # now; the DVE muls are deferred so they don't block the
            # attention-critical DVE stream behind ACT copy latency.
            raw = ropep.tile([128, CH], F16, tag="raw", name=f"raw{c}_{ft}")
            nc.scalar.copy(raw[:], ps[:])
            rot = ropep.tile([128, CH], F16, tag="rot", name=f"rot{c}_{ft}")
            nc.sync.dma_start(rot[0:64, :], raw[64:128, :])
            nc.sync.dma_start(rot[64:128, :], raw[0:64, :])
            deferred_rope.append((raw, rot, ft, c))

        def flush_rope():
            for raw, rot, ft, c in deferred_rope:
                h = ft % 2
                ta = ropep.tile([128, CH], F16, tag="ta", name=f"ta{c}_{ft}")
                tb = ropep.tile([128, CH], F16, tag="tb", name=f"tb{c}_{ft}")
                csl = slice(c * CH, (c + 1) * CH)
                nc.vector.tensor_mul(ta[:], raw[:], cos_sb[:, csl])
                nc.vector.tensor_mul(tb[:], rot[:], sin_sb[:, csl])
                dst = QTR[h][c] if ft < 2 else KTR[h][c]
                nc.vector.tensor_add(dst[:], ta[:], tb[:])
            deferred_rope.clear()

        def qk_chain(c, ft):
            """Q^T/K^T feature tile ft of chunk c: fp8 split DoubleRow."""
            hh, hl = hid_tiles[c]
            ps = psqkv.tile([128, CH], F32, tag="qkv", name=f"psqkv{c}_{ft}")
            fsl = slice(ft * 128, (ft + 1) * 128)
            i, n = 0, 3 * NKP
            for wt, xt in ((wqh, hh), (wqh, hl), (wql, hh)):
                for kp in range(NKP):
                    ksl = slice(2 * kp, 2 * kp + 2)
                    nc.tensor.matmul(ps[:], wt[:, ksl, fsl], xt[:, ksl, :],
                                     start=(i == 0), stop=(i == n - 1),
                                     perf_mode=DR)
                    i += 1
            rope_evict(ps, ft, c)

        def v_chain(c, j):
            """V rows for t-tile 4c+j in [t, d] layout: fp8 split DoubleRow."""
            hh, hl = hid_tiles[c]
            ps = psqkv.tile([128, CH], F32, tag="qkv", name=f"psv{c}_{j}")
            jsl = slice(j * 128, (j + 1) * 128)
            vsl = slice(2 * FEAT, 3 * FEAT)
            i, n = 0, 3 * NKP
            for wt, xt in ((wqh, hh), (wqh, hl), (wql, hh)):
                for kp in range(NKP):
                    ksl = slice(2 * kp, 2 * kp + 2)
                    nc.tensor.matmul(ps[:, :FEAT],
                                     xt[:, ksl, jsl], wt[:, ksl, vsl],
                                     start=(i == 0), stop=(i == n - 1),
                                     perf_mode=DR)
                    i += 1
            kt_ = 4 * c + j
            nc.scalar.copy(VV[:, kt_ * FEAT:(kt_ + 1) * FEAT], ps[:, :FEAT])

        # ---------------- attention for one (chunk, head) ----------------
        def attn_head(c, h):
            nkt = 4 * (c + 1)
            po = pso.tile([128, CH], F32, tag="o", name=f"po{h}_{c}")
            # 3 fp16 accumulators for the softmax denominator: A/B on DVE,
            # C on Pool (scalar_tensor_tensor), bounding rounding depth
            acc = [accp.tile([128, CH], F16, tag="acc", name=f"acc{h}_{c}_{i}")
                   for i in range(3)]
            first = [True, True, True]
            if c == 0:
                for a in acc:
                    nc.any.memset(a[:], 0.0)
                    first = [False, False, False]
            else:
                nc.any.memset(acc[2][:], 0.0)
                first[2] = False

            def s_exp(kt):
                r = kt - 4 * c
                qo = 128 * r if r > 0 else 0
                ps = pss.tile([128, CH], F32, tag="s", name=f"pss{h}_{c}_{kt}")
                nc.tensor.matmul(
                    ps[:, qo:],
                    KTR[h][kt // 4][:, (kt % 4) * 128:(kt % 4 + 1) * 128],
                    QTR[h][c][:, qo:],
                    start=True, stop=True)
                if r >= 0:
                    nc.vector.tensor_add(
                        ps[:, qo:qo + 128], ps[:, qo:qo + 128],
                        mask_sb[:, r * CH + qo:r * CH + qo + 128])
                pt = ptp.tile([128, CH], F16, tag="pt", name=f"pt{h}_{c}_{kt}")
                nc.scalar.activation(pt[:, qo:], ps[:, qo:],
                                     mybir.ActivationFunctionType.Exp,
                                     scale=SCALE / (WSCALE * WSCALE))
                return kt, qo, pt

            def pv_acc(pend):
                kt, qo, pt = pend
                nc.tensor.matmul(
                    po[:, qo:],
                    VV[:, kt * FEAT + h * 128:kt * FEAT + (h + 1) * 128],
                    pt[:, qo:],
                    start=(kt == 0), stop=(kt == nkt - 1))
                i = kt % 3
                if i == 2:
                    nc.gpsimd.scalar_tensor_tensor(
                        acc[2][:, qo:], pt[:, qo:], 1.0, acc[2][:, qo:],
                        op0=MUL, op1=ADD)
                elif first[i]:
                    nc.vector.tensor_copy(acc[i][:], pt[:])
                    first[i] = False
                else:
                    nc.vector.tensor_add(acc[i][:, qo:], acc[i][:, qo:],
                                         pt[:, qo:])

            pending = []
            for kt in range(nkt):
                pending.append(s_exp(kt))
                if len(pending) > 2:
                    pv_acc(pending.pop(0))
            for pend in pending:
                pv_acc(pend)

            # denominator + normalize + fp8 split
            nc.vector.tensor_add(acc[0][:], acc[0][:], acc[1][:])
            nc.vector.tensor_add(acc[0][:], acc[0][:], acc[2][:])
            pdall = nrmp.tile([128, CH], F32, tag="pdall", name=f"pd{h}_{c}")
            nc.gpsimd.partition_all_reduce(pdall[:], acc[0][:], channels=128,
                                           reduce_op=bass_isa.ReduceOp.add)
            binv = nrmp.tile([128, CH], F32, tag="binv", name=f"bi{h}_{c}")
            nc.vector.reciprocal(binv[:], pdall[:])
            at16 = nrmp.tile([128, CH], F16, tag="at16", name=f"a16{h}_{c}")
            nc.vector.scalar_tensor_tensor(
                at16[:], po[:], ATSCALE / WSCALE, binv[:], op0=MUL, op1=MUL)
            nc.vector.tensor_copy(ATH[c][:, h, :], at16[:])
            nc.vector.tensor_sub(ATL[c][:, h, :], at16[:], ATH[c][:, h, :])

        # ---------------- w_o for one T-tile (fp8 split DoubleRow) --------
        def wo_tile(c, j, dve_stg):
            tt = 4 * c + j
            stg = stgp.tile([128, HID], F16, tag="stg", name=f"stg{tt}")
            jsl = slice(j * 128, (j + 1) * 128)
            for n in range(HID // CH):
                nsl = slice(n * CH, (n + 1) * CH)
                pw = pswo.tile([128, CH], F32, tag="w", name=f"pw{tt}_{n}")
                for i, (a, w) in enumerate(
                        ((ATH[c], woh), (ATH[c], wol), (ATL[c], woh))):
                    nc.tensor.matmul(pw[:], a[:, :, jsl], w[:, :, nsl],
                                     start=(i == 0), stop=(i == 2),
                                     perf_mode=DR)
                if (n + tt) % 4 < dve_stg:
                    nc.vector.tensor_copy(stg[:, nsl], pw[:])
                else:
                    nc.scalar.copy(stg[:, nsl], pw[:])
            eng = nc.sync if tt % 2 == 0 else nc.scalar
            eng.dma_start(out[tt * 128:(tt + 1) * 128, :], stg[:])

        # ================= main schedule =================
        # stage 0: load + QKV(0); stage k: QKV(k) + attn(k-1) + wo(k-2)
        load_hid(0)
        load_hid(1)
        for ft in range(4):
            qk_chain(0, ft)
        flush_rope()
        for j in range(4):
            v_chain(0, j)

        for k in range(1, NCH + 1):
            c_attn = k - 1
            c_wo = k - 2
            if k < NCH:
                if k + 1 < NCH:
                    load_hid(k + 1)
                attn_head(c_attn, 0)
                qk_chain(k, 0)
                qk_chain(k, 2)
                attn_head(c_attn, 1)
                for j in range(4):
                    v_chain(k, j)
                qk_chain(k, 1)
                qk_chain(k, 3)
                flush_rope()
            else:
                attn_head(c_attn, 0)
                attn_head(c_attn, 1)
            if c_wo >= 0:
                for j in range(4):
                    wo_tile(c_wo, j, dve_stg=2)
        for j in range(4):
            wo_tile(NCH - 1, j, dve_stg=2)


_NC_CACHE = None


def _get_nc():
    global _NC_CACHE
    if _NC_CACHE is None:
        _NC_CACHE = _build_bass()
    return _NC_CACHE


def _split8(x):
    hi = np.ascontiguousarray(x).astype(NE4)
    lo = (x - hi.astype(np.float32)).astype(NE5)
    return hi, np.ascontiguousarray(lo)


def prepare_inputs(hidden_states, positions, w_qkv, w_o):
    """Host-side sharding/preprocessing -> list of per-core input maps."""
    hidden_states = np.asarray(hidden_states, dtype=np.float32)
    positions = np.asarray(positions)
    w_qkv = np.asarray(w_qkv, dtype=np.float32)
    w_o = np.asarray(w_o, dtype=np.float32)

    # hidden^T in [128, KT, T] k-tile layout, fp8 hi/lo
    hidT = hidden_states.T.reshape(KT, 128, T).transpose(1, 0, 2)
    hid_hi, hid_lo = _split8(hidT)

    pos = positions.astype(np.float32)
    half = HD // 2
    inv_freq = 1.0 / (10000.0 ** (np.arange(half, dtype=np.float32) / half))
    freqs = np.outer(pos, inv_freq)          # [T, 64]
    cos = np.cos(freqs).T                    # [64, T]
    sin = np.sin(freqs).T
    cos2 = np.concatenate([cos, cos], axis=0).astype(np.float16)
    sin2 = np.concatenate([-sin, sin], axis=0).astype(np.float16)

    k_idx = np.arange(128)[:, None]
    q_idx = np.arange(CH)[None, :]
    mblocks = [np.where(128 * r + k_idx <= q_idx, 0.0, MASK_NEG).astype(np.float32)
               for r in range(4)]
    masks_np = np.concatenate(mblocks, axis=1)

    in_maps = []
    for core in range(NCORES):
        heads = [HPC * core + i for i in range(HPC)]
        wq = [w_qkv[:, h * HD:(h + 1) * HD] for h in heads]
        wk = [w_qkv[:, FEAT * NCORES + h * HD:FEAT * NCORES + (h + 1) * HD]
              for h in heads]
        wv = [w_qkv[:, 2 * FEAT * NCORES + h * HD:2 * FEAT * NCORES + (h + 1) * HD]
              for h in heads]
        wqkv_core = np.concatenate(wq + wk + wv, axis=1) * WSCALE
        wqkv_core = wqkv_core.reshape(KT, 128, QKVF).transpose(1, 0, 2)
        wq_hi, wq_lo = _split8(wqkv_core)
        wo_core = np.stack(
            [w_o[h * HD:(h + 1) * HD, :] for h in heads], axis=0) * WSCALE
        wo_core = wo_core.transpose(1, 0, 2)   # [128, HPC, HID]
        wo_hi, wo_lo = _split8(wo_core)
        in_maps.append({
            "hid_hi": hid_hi,
            "hid_lo": hid_lo,
            "wq_hi": wq_hi,
            "wq_lo": wq_lo,
            "wo_hi": wo_hi,
            "wo_lo": wo_lo,
            "cos2": cos2,
            "sin2": sin2,
            "masks": masks_np,
        })
    return in_maps


def kernel(hidden_states, positions, w_qkv, w_o):
    in_maps = prepare_inputs(hidden_states, positions, w_qkv, w_o)
    nc = _get_nc()
    try:
        res = run_bass_kernel_spmd(nc, in_maps, core_ids=list(range(NCORES)))
    except Exception:
        # transient device wedge from a prior crashed process: retry once
        res = run_bass_kernel_spmd(nc, in_maps, core_ids=list(range(NCORES)))
    acc = res.results[0]["out"].astype(np.float32)
    for i in range(1, NCORES):
        acc += res.results[i]["out"].astype(np.float32)
    return acc * (1.0 / (ATSCALE * WSCALE))


# revision 6
# speedup vs baseline: 1.1845x; 1.0011x over previous
"""Trainium2 Bass kernel for DeepseekAttention (T=4096, H=2048, 16 heads, d=128).

Tensor-parallel over heads: 8 NeuronCores x 2 heads each (SPMD).

v1 design (fp8 DoubleRow + full phase interleaving):
  - QKV projection and w_o projection run as split-precision fp8 matmuls
    (hi=e4m3, lo=e5m2, 3 cross terms, lo*lo dropped) in DoubleRow perf mode,
    which contracts 2 k-tiles per instruction at 0.5 cycles/row: 0.75x the
    fp16 PE cost at ~0.2% error. Weights are pre-scaled by 32 on the host to
    center them in e4m3's range; the scale is undone via the exp() scale
    (Q.K picks up 32*32=1024) and a final host-side 1/256 on the output.
  - Attention S/PV matmuls stay fp16 (accuracy requires it).
  - Softmax denominator: exp tiles are accumulated on DVE/Pool into 3 fp16
    accumulators (bounded rounding depth), reduced across partitions with
    gpsimd partition_all_reduce -- no PE ones-matmuls, no extra PSUM bank.
  - Attention output is normalized and split to (e4m3 hi, e5m2 lo) on the
    fly for the fp8 w_o pass.
  - QKV projection for chunk c+1 is interleaved with attention for chunk c
    and w_o for chunk c-1, keeping PE continuously busy (p-state stays at
    2.4 GHz) and hiding ACT exp / DVE work under PE matmuls.
"""

import numpy as np
import ml_dtypes

import concourse.tile as tile
from concourse import bacc, bass_isa, mybir
from concourse.bass_utils import run_bass_kernel_spmd

T = 4096
HID = 2048
NHEADS = 16
HD = 128
NCORES = 8
HPC = NHEADS // NCORES        # 2 heads per core
FEAT = HPC * HD               # 256 per-core attention features
QKVF = 3 * FEAT               # 768 per-core qkv features
CH = 512                      # T-chunk width
NCH = T // CH                 # 8 chunks
KT = HID // 128               # 16 hidden k-tiles
NKP = KT // 2                 # 8 DoubleRow k-pairs
SCALE = float(HD) ** -0.5
WSCALE = 32.0                 # host pre-scale on w_qkv and w_o (e4m3 range)
ATSCALE = 8.0                 # scale on normalized attention output
MASK_NEG = -3.0e7             # scores carry a 1024x factor; must still kill exp

BF = mybir.dt.bfloat16
F16 = mybir.dt.float16
F32 = mybir.dt.float32
E4 = mybir.dt.float8e4
E5 = mybir.dt.float8e5
NE4 = ml_dtypes.float8_e4m3
NE5 = ml_dtypes.float8_e5m2
DR = mybir.MatmulPerfMode.DoubleRow
MUL = mybir.AluOpType.mult
ADD = mybir.AluOpType.add


def _build_bass():
    nc = bacc.Bacc("TRN2", target_bir_lowering=False, debug=False,
                   num_devices=NCORES)

    hid_hi = nc.dram_tensor("hid_hi", [128, KT, T], E4, kind="ExternalInput").ap()
    hid_lo = nc.dram_tensor("hid_lo", [128, KT, T], E5, kind="ExternalInput").ap()
    wq_hi = nc.dram_tensor("wq_hi", [128, KT, QKVF], E4, kind="ExternalInput").ap()
    wq_lo = nc.dram_tensor("wq_lo", [128, KT, QKVF], E5, kind="ExternalInput").ap()
    wo_hi = nc.dram_tensor("wo_hi", [128, HPC, HID], E4, kind="ExternalInput").ap()
    wo_lo = nc.dram_tensor("wo_lo", [128, HPC, HID], E5, kind="ExternalInput").ap()
    cos2 = nc.dram_tensor("cos2", [128, T], F16, kind="ExternalInput").ap()
    sin2 = nc.dram_tensor("sin2", [128, T], F16, kind="ExternalInput").ap()
    masks = nc.dram_tensor("masks", [128, 4 * CH], F32, kind="ExternalInput").ap()
    out = nc.dram_tensor("out", [T, HID], F16, kind="ExternalOutput").ap()

    with tile.TileContext(nc) as tc:
        _emit(tc, hid_hi, hid_lo, wq_hi, wq_lo, wo_hi, wo_lo, cos2, sin2,
              masks, out)
    nc.compile()
    return nc


def _emit(tc, hid_hi, hid_lo, wq_hi, wq_lo, wo_hi, wo_lo, cos2, sin2,
          masks, out):
    nc = tc.nc
    from contextlib import ExitStack
    ctx = ExitStack()
    with ctx:
        const = ctx.enter_context(tc.tile_pool(name="const", bufs=1))
        hidp = ctx.enter_context(tc.tile_pool(name="hidp", bufs=2))
        ropep = ctx.enter_context(tc.tile_pool(name="ropep", bufs=4))
        persist = ctx.enter_context(tc.tile_pool(name="persist", bufs=1))
        ptp = ctx.enter_context(tc.tile_pool(name="ptp", bufs=8))
        accp = ctx.enter_context(tc.tile_pool(name="accp", bufs=6))
        nrmp = ctx.enter_context(tc.tile_pool(name="nrmp", bufs=2))
        stgp = ctx.enter_context(tc.tile_pool(name="stgp", bufs=2))
        # PSUM: 8 banks: qkv 2, S 2, po 2, wo 2
        psqkv = ctx.enter_context(tc.tile_pool(name="psqkv", bufs=2, space="PSUM"))
        pss = ctx.enter_context(tc.tile_pool(name="pss", bufs=2, space="PSUM"))
        pso = ctx.enter_context(tc.tile_pool(name="pso", bufs=2, space="PSUM"))
        pswo = ctx.enter_context(tc.tile_pool(name="pswo", bufs=2, space="PSUM"))

        # ---- constants (scalar queue) ----
        wqh = const.tile([128, KT, QKVF], E4, tag="wqh")
        wql = const.tile([128, KT, QKVF], E5, tag="wql")
        nc.scalar.dma_start(wqh[:], wq_hi[:])
        nc.scalar.dma_start(wql[:], wq_lo[:])
        woh = const.tile([128, HPC, HID], E4, tag="woh")
        wol = const.tile([128, HPC, HID], E5, tag="wol")
        nc.scalar.dma_start(woh[:], wo_hi[:])
        nc.scalar.dma_start(wol[:], wo_lo[:])
        cos_sb = const.tile([128, T], F16, tag="cos_sb")
        sin_sb = const.tile([128, T], F16, tag="sin_sb")
        mask_sb = const.tile([128, 4 * CH], F32, tag="mask_sb")
        nc.scalar.dma_start(cos_sb[:], cos2[:])
        nc.scalar.dma_start(sin_sb[:], sin2[:])
        nc.scalar.dma_start(mask_sb[:], masks[:])

        # ---- persistent activation tiles ----
        QTR = [[persist.tile([128, CH], F16, tag=f"qtr{h}_{c}", name=f"qtr{h}_{c}")
                for c in range(NCH)] for h in range(HPC)]
        KTR = [[persist.tile([128, CH], F16, tag=f"ktr{h}_{c}", name=f"ktr{h}_{c}")
                for c in range(NCH)] for h in range(HPC)]
        VV = persist.tile([128, HPC * T], F16, tag="vv", name="vv")
        ATH = [persist.tile([128, HPC, CH], E4, tag=f"ath{c}", name=f"ath{c}")
               for c in range(NCH)]
        ATL = [persist.tile([128, HPC, CH], E5, tag=f"atl{c}", name=f"atl{c}")
               for c in range(NCH)]

        hid_tiles = {}

        def load_hid(c):
            hh = hidp.tile([128, KT, CH], E4, tag="hh", name=f"hh{c}")
            hl = hidp.tile([128, KT, CH], E5, tag="hl", name=f"hl{c}")
            csl = slice(c * CH, (c + 1) * CH)
            nc.gpsimd.dma_start(hh[:], hid_hi[:, :, csl])
            nc.gpsimd.dma_start(hl[:], hid_lo[:, :, csl])
            hid_tiles[c] = (hh, hl)

        deferred_rope = []

        def rope_evict(ps, ft, c):
            # ft: 0=q.h0 1=q.h1 2=k.h0 3=k.h1. ACT copy + partition-swap DMA
            # now; the DVE muls are deferred so they don't block the
            # attention-critical DVE stream behind ACT copy latency.
            raw = ropep.tile([128, CH], F16, tag="raw", name=f"raw{c}_{ft}")
            nc.scalar.copy(raw[:], ps[:])
            rot = ropep.tile([128, CH], F16, tag="rot", name=f"rot{c}_{ft}")
            nc.sync.dma_start(rot[0:64, :], raw[64:128, :])
            nc.sync.dma_start(rot[64:128, :], raw[0:64, :])
            deferred_rope.append((raw, rot, ft, c))

        def flush_rope():
            for raw, rot, ft, c in deferred_rope:
                h = ft % 2
                ta = ropep.tile([128, CH], F16, tag="ta", name=f"ta{c}_{ft}")
                tb = ropep.tile([128, CH], F16, tag="tb", name=f"tb{c}_{ft}")
                csl = slice(c * CH, (c + 1) * CH)
                nc.vector.tensor_mul(ta[:], raw[:], cos_sb[:, csl])
                nc.vector.tensor_mul(tb[:], rot[:], sin_sb[:, csl])
                dst = QTR[h][c] if ft < 2 else KTR[h][c]
                nc.vector.tensor_add(dst[:], ta[:], tb[:])
            deferred_rope.clear()

        def qk_chain(c, ft):
            """Q^T/K^T feature tile ft of chunk c: fp8 split DoubleRow."""
            hh, hl = hid_tiles[c]
            ps = psqkv.tile([128, CH], F32, tag="qkv", name=f"psqkv{c}_{ft}")
            fsl = slice(ft * 128, (ft + 1) * 128)
            i, n = 0, 3 * NKP
            for wt, xt in ((wqh, hh), (wqh, hl), (wql, hh)):
                for kp in range(NKP):
                    ksl = slice(2 * kp, 2 * kp + 2)
                    nc.tensor.matmul(ps[:], wt[:, ksl, fsl], xt[:, ksl, :],
                                     start=(i == 0), stop=(i == n - 1),
                                     perf_mode=DR)
                    i += 1
            rope_evict(ps, ft, c)

        def v_chain(c, j):
            """V rows for t-tile 4c+j in [t, d] layout: fp8 split DoubleRow."""
            hh, hl = hid_tiles[c]
            ps = psqkv.tile([128, CH], F32, tag="qkv", name=f"psv{c}_{j}")
            jsl = slice(j * 128, (j + 1) * 128)
            vsl = slice(2 * FEAT, 3 * FEAT)
            i, n = 0, 3 * NKP
            for wt, xt in ((wqh, hh), (wqh, hl), (wql, hh)):
                for kp in range(NKP):
                    ksl = slice(2 * kp, 2 * kp + 2)
                    nc.tensor.matmul(ps[:, :FEAT],
                                     xt[:, ksl, jsl], wt[:, ksl, vsl],
                                     start=(i == 0), stop=(i == n - 1),
                                     perf_mode=DR)
                    i += 1
            kt_ = 4 * c + j
            nc.scalar.copy(VV[:, kt_ * FEAT:(kt_ + 1) * FEAT], ps[:, :FEAT])

        # ---------------- attention for one (chunk, head) ----------------
        def attn_head(c, h):
            nkt = 4 * (c + 1)
            po = pso.tile([128, CH], F32, tag="o", name=f"po{h}_{c}")
            # 3 fp16 accumulators for the softmax denominator: A/B on DVE,
            # C on Pool (scalar_tensor_tensor), bounding rounding depth
            acc = [accp.tile([128, CH], F16, tag="acc", name=f"acc{h}_{c}_{i}")
                   for i in range(3)]
            first = [True, True, True]
            if c == 0:
                for a in acc:
                    nc.any.memset(a[:], 0.0)
                first = [False, False, False]

            def s_exp(kt):
                r = kt - 4 * c
                qo = 128 * r if r > 0 else 0
                ps = pss.tile([128, CH], F32, tag="s", name=f"pss{h}_{c}_{kt}")
                nc.tensor.matmul(
                    ps[:, qo:],
                    KTR[h][kt // 4][:, (kt % 4) * 128:(kt % 4 + 1) * 128],
                    QTR[h][c][:, qo:],
                    start=True, stop=True)
                if r >= 0:
                    nc.vector.tensor_add(
                        ps[:, qo:qo + 128], ps[:, qo:qo + 128],
                        mask_sb[:, r * CH + qo:r * CH + qo + 128])
                pt = ptp.tile([128, CH], F16, tag="pt", name=f"pt{h}_{c}_{kt}")
                nc.scalar.activation(pt[:, qo:], ps[:, qo:],
                                     mybir.ActivationFunctionType.Exp,
                                     scale=SCALE / (WSCALE * WSCALE))
                return kt, qo, pt

            def pv_acc(pend):
                kt, qo, pt = pend
                nc.tensor.matmul(
                    po[:, qo:],
                    VV[:, kt * FEAT + h * 128:kt * FEAT + (h + 1) * 128],
                    pt[:, qo:],
                    start=(kt == 0), stop=(kt == nkt - 1))
                i = kt % 3
                if first[i]:
                    nc.vector.tensor_copy(acc[i][:], pt[:])
                    first[i] = False
                else:
                    nc.vector.tensor_add(acc[i][:, qo:], acc[i][:, qo:],
                                         pt[:, qo:])

            pending = []
            for kt in range(nkt):
                pending.append(s_exp(kt))
                if len(pending) > 2:
                    pv_acc(pending.pop(0))
            for pend in pending:
                pv_acc(pend)

            # denominator + normalize + fp8 split
            nc.vector.tensor_add(acc[0][:], acc[0][:], acc[1][:])
            nc.vector.tensor_add(acc[0][:], acc[0][:], acc[2][:])
            pdall = nrmp.tile([128, CH], F32, tag="pdall", name=f"pd{h}_{c}")
            nc.gpsimd.partition_all_reduce(pdall[:], acc[0][:], channels=128,
                                           reduce_op=bass_isa.ReduceOp.add)
            binv = nrmp.tile([128, CH], F32, tag="binv", name=f"bi{h}_{c}")
            nc.vector.reciprocal(binv[:], pdall[:])
            at16 = nrmp.tile([128, CH], F16, tag="at16", name=f"a16{h}_{c}")
            nc.vector.scalar_tensor_tensor(
                at16[:], po[:], ATSCALE / WSCALE, binv[:], op0=MUL, op1=MUL)
            nc.vector.tensor_copy(ATH[c][:, h, :], at16[:])
            nc.vector.tensor_sub(ATL[c][:, h, :], at16[:], ATH[c][:, h, :])

        # ---------------- w_o for one T-tile (fp8 split DoubleRow) --------
        def wo_tile(c, j, dve_stg):
            tt = 4 * c + j
            stg = stgp.tile([128, HID], F16, tag="stg", name=f"stg{tt}")
            jsl = slice(j * 128, (j + 1) * 128)
            for n in range(HID // CH):
                nsl = slice(n * CH, (n + 1) * CH)
                pw = pswo.tile([128, CH], F32, tag="w", name=f"pw{tt}_{n}")
                for i, (a, w) in enumerate(
                        ((ATH[c], woh), (ATH[c], wol), (ATL[c], woh))):
                    nc.tensor.matmul(pw[:], a[:, :, jsl], w[:, :, nsl],
                                     start=(i == 0), stop=(i == 2),
                                     perf_mode=DR)
                if (n + tt) % 4 < dve_stg:
                    nc.vector.tensor_copy(stg[:, nsl], pw[:])
                else:
                    nc.scalar.copy(stg[:, nsl], pw[:])
            eng = nc.sync if tt % 2 == 0 else nc.scalar
            eng.dma_start(out[tt * 128:(tt + 1) * 128, :], stg[:])

        # ================= main schedule =================
        # stage 0: load + QKV(0); stage k: QKV(k) + attn(k-1) + wo(k-2)
        load_hid(0)
        load_hid(1)
        for ft in range(4):
            qk_chain(0, ft)
        flush_rope()
        for j in range(4):
            v_chain(0, j)

        for k in range(1, NCH + 1):
            c_attn = k - 1
            c_wo = k - 2
            if k < NCH:
                if k + 1 < NCH:
                    load_hid(k + 1)
                attn_head(c_attn, 0)
                qk_chain(k, 0)
                qk_chain(k, 2)
                attn_head(c_attn, 1)
                for j in range(4):
                    v_chain(k, j)
                qk_chain(k, 1)
                qk_chain(k, 3)
                flush_rope()
            else:
                attn_head(c_attn, 0)
                attn_head(c_attn, 1)
            if c_wo >= 0:
                for j in range(4):
                    wo_tile(c_wo, j, dve_stg=2)
        for j in range(4):
            wo_tile(NCH - 1, j, dve_stg=2)


_NC_CACHE = None


def _get_nc():
    global _NC_CACHE
    if _NC_CACHE is None:
        _NC_CACHE = _build_bass()
    return _NC_CACHE


def _split8(x):
    hi = np.ascontiguousarray(x).astype(NE4)
    lo = (x - hi.astype(np.float32)).astype(NE5)
    return hi, np.ascontiguousarray(lo)


def prepare_inputs(hidden_states, positions, w_qkv, w_o):
    """Host-side sharding/preprocessing -> list of per-core input maps."""
    hidden_states = np.asarray(hidden_states, dtype=np.float32)
    positions = np.asarray(positions)
    w_qkv = np.asarray(w_qkv, dtype=np.float32)
    w_o = np.asarray(w_o, dtype=np.float32)

    # hidden^T in [128, KT, T] k-tile layout, fp8 hi/lo
    hidT = hidden_states.T.reshape(KT, 128, T).transpose(1, 0, 2)
    hid_hi, hid_lo = _split8(hidT)

    pos = positions.astype(np.float32)
    half = HD // 2
    inv_freq = 1.0 / (10000.0 ** (np.arange(half, dtype=np.float32) / half))
    freqs = np.outer(pos, inv_freq)          # [T, 64]
    cos = np.cos(freqs).T                    # [64, T]
    sin = np.sin(freqs).T
    cos2 = np.concatenate([cos, cos], axis=0).astype(np.float16)
    sin2 = np.concatenate([-sin, sin], axis=0).astype(np.float16)

    k_idx = np.arange(128)[:, None]
    q_idx = np.arange(CH)[None, :]
    mblocks = [np.where(128 * r + k_idx <= q_idx, 0.0, MASK_NEG).astype(np.float32)
               for r in range(4)]
    masks_np = np.concatenate(mblocks, axis=1)

    in_maps = []
    for core in range(NCORES):
        heads = [HPC * core + i for i in range(HPC)]
        wq = [w_qkv[:, h * HD:(h + 1) * HD] for h in heads]
        wk = [w_qkv[:, FEAT * NCORES + h * HD:FEAT * NCORES + (h + 1) * HD]
              for h in heads]
        wv = [w_qkv[:, 2 * FEAT * NCORES + h * HD:2 * FEAT * NCORES + (h + 1) * HD]
              for h in heads]
        wqkv_core = np.concatenate(wq + wk + wv, axis=1) * WSCALE
        wqkv_core = wqkv_core.reshape(KT, 128, QKVF).transpose(1, 0, 2)
        wq_hi, wq_lo = _split8(wqkv_core)
        wo_core = np.stack(
            [w_o[h * HD:(h + 1) * HD, :] for h in heads], axis=0) * WSCALE
        wo_core = wo_core.transpose(1, 0, 2)   # [128, HPC, HID]
        wo_hi, wo_lo = _split8(wo_core)
        in_maps.append({
            "hid_hi": hid_hi,
            "hid_lo": hid_lo,
            "wq_hi": wq_hi,
            "wq_lo": wq_lo,
            "wo_hi": wo_hi,
            "wo_lo": wo_lo,
            "cos2": cos2,
            "sin2": sin2,
            "masks": masks_np,
        })
    return in_maps


def kernel(hidden_states, positions, w_qkv, w_o):
    in_maps = prepare_inputs(hidden_states, positions, w_qkv, w_o)
    nc = _get_nc()
    try:
        res = run_bass_kernel_spmd(nc, in_maps, core_ids=list(range(NCORES)))
    except Exception:
        # transient device wedge from a prior crashed process: retry once
        res = run_bass_kernel_spmd(nc, in_maps, core_ids=list(range(NCORES)))
    acc = res.results[0]["out"].astype(np.float32)
    for i in range(1, NCORES):
        acc += res.results[i]["out"].astype(np.float32)
    return acc * (1.0 / (ATSCALE * WSCALE))
